# revision 3
# baseline (speedup 1.0000x reference)
"""Causal self-attention kernel for Trainium2, 8-core data parallel.

Per-core program: one batch element b of x [8, 1024, 768].
  qk^T = (x W_qk^T + b)^T  (features on partitions)
  per head: S^T = K^T.T-slices @ Q^T  (k on partitions, q free),
            P^T = exp(S^T/8) with causal tri-mask on diagonal blocks,
            y^T_h = V_h.T-slices @ P^T with concurrent col-tiled
            ones-matmul accumulating softmax denominators,
            normalize via broadcast reciprocal.
  out = y W_p^T + b_p  (t on partitions).
All matmuls bf16 inputs / fp32 PSUM accumulation; softmax in fp32.
"""
import sys
import contextlib
from contextlib import ExitStack

sys.path.insert(0, "/opt/trn_rl_repo")

import numpy as np

import concourse.bass as bass
import concourse.bacc as bacc
import concourse.mybir as mybir
import concourse.tile as tile
from concourse.masks import make_identity, make_upper_triangular

F32 = mybir.dt.float32
BF16 = mybir.dt.bfloat16

P = 128
T = 1024
C = 768
H = 12
HS = 64
CT = C // P     # 6 c-tiles
TT = T // P     # 8 t-tiles
KT = T // P     # 8 k-tiles per head
N_CORES = 8


def emit_consts(nc, tc, const, ba_d, bp_d):
    ident = const.tile([P, P], F32, tag="ident")
    make_identity(nc, ident)
    tri = const.tile([P, P], BF16, tag="tri")
    make_upper_triangular(nc, tri, val=1.0, diag=True)
    ones_bf = const.tile([P, P], BF16, tag="ones")
    nc.gpsimd.memset(ones_bf[:], 1.0)
    # per-partition bias for the q/k features (j on partitions)
    bias_qk = const.tile([P, 12], F32, tag="bqk")
    nc.sync.dma_start(bias_qk[:], ba_d[0 : 2 * C].rearrange("(o p) -> p o", p=P))
    # broadcast-to-all-partitions copies of the V bias and proj bias (j on free dim)
    Bv = const.tile([P, C], F32, tag="Bv")
    nc.sync.dma_start(
        Bv[:],
        ba_d[2 * C : 3 * C].rearrange("(a j) -> a j", a=1).to_broadcast([P, C]),
    )
    Bp = const.tile([P, C], F32, tag="Bp")
    nc.sync.dma_start(
        Bp[:], bp_d.rearrange("(a j) -> a j", a=1).to_broadcast([P, C])
    )
    return dict(ident=ident, tri=tri, ones_bf=ones_bf, bias_qk=bias_qk, Bv=Bv, Bp=Bp)


def emit_body(nc, tc, pools, cst, x_d, wa_d, wp_d, y_d):
    const, persist, nat, work, ptp, ps512, ps384 = pools
    ident, tri, ones_bf = cst["ident"], cst["tri"], cst["ones_bf"]
    bias_qk, Bv, Bp = cst["bias_qk"], cst["Bv"], cst["Bp"]

    xT = persist.tile([P, CT, T], BF16, tag="xT")
    WT = persist.tile([P, CT, 3 * C], BF16, tag="WT")
    WpT = persist.tile([P, CT, C], BF16, tag="WpT")
    qkT = persist.tile([P, 12, T], BF16, tag="qkT")
    V = persist.tile([P, TT, C], BF16, tag="V")
    yT = persist.tile([P, CT, T], BF16, tag="yT")

    # ---- phase 0: transpose x, W_attn, W_proj into c-on-partitions layouts
    def transpose_in(dram_ap, n_row_tiles, dst):
        # dram_ap: [n_row_tiles*128, C] f32, row-major; dst: [P, CT, n_row_tiles*128] bf16
        src = dram_ap.rearrange("(rt p) c -> rt p c", p=P)
        for rt in range(n_row_tiles):
            natt = nat.tile([P, C], F32, tag="nat")
            nc.sync.dma_start(natt[:], src[rt])
            for g in range(2):  # two groups of 3 c-tiles
                ps = ps384.tile([P, 384], F32, tag="ps384")
                for i in range(3):
                    ct = g * 3 + i
                    nc.tensor.transpose(
                        ps[:, i * P : (i + 1) * P],
                        natt[:, ct * P : (ct + 1) * P],
                        ident[:],
                    )
                nc.vector.tensor_copy(
                    dst[:, g * 3 : (g + 1) * 3, rt * P : (rt + 1) * P],
                    ps[:].rearrange("p (a b) -> p a b", a=3),
                )

    transpose_in(wa_d, 18, WT)
    transpose_in(x_d, TT, xT)
    transpose_in(wp_d, CT, WpT)

    # ---- phase 1: qkT[j, t] for q/k features; V[t, j] for v features
    for jt in range(12):
        for tb in range(2):
            ps = ps512.tile([P, 512], F32, tag="ps512")
            for ct in range(CT):
                nc.tensor.matmul(
                    ps[:],
                    WT[:, ct, jt * P : (jt + 1) * P],
                    xT[:, ct, tb * 512 : (tb + 1) * 512],
                    start=(ct == 0),
                    stop=(ct == CT - 1),
                )
            nc.vector.tensor_scalar_add(
                qkT[:, jt, tb * 512 : (tb + 1) * 512], ps[:], bias_qk[:, jt : jt + 1]
            )
    for tt in range(TT):
        for jb in range(2):
            ps = ps384.tile([P, 384], F32, tag="ps384")
            for ct in range(CT):
                nc.tensor.matmul(
                    ps[:],
                    xT[:, ct, tt * P : (tt + 1) * P],
                    WT[:, ct, 2 * C + jb * 384 : 2 * C + (jb + 1) * 384],
                    start=(ct == 0),
                    stop=(ct == CT - 1),
                )
            nc.vector.tensor_add(
                V[:, tt, jb * 384 : (jb + 1) * 384],
                ps[:],
                Bv[:, jb * 384 : (jb + 1) * 384],
            )

    # ---- phase 2: per-head attention
    for h in range(H):
        hb = 64 * (h % 2)       # partition base of this head's rows in qkT / yT
        vb = hb                 # psum col base for the PV output
        db = 64 - hb            # psum col base for the denominator column
        qj = h // 2             # qkT tile index of Q features
        kj = 6 + h // 2         # qkT tile index of K features

        PT = ptp.tile([P, KT, T], BF16, tag="PT")
        # S^T = K_kt^T.T @ Q^T ; exp ; tri-mask on diagonal 128-block
        for kt in range(KT):
            qs = kt * P
            chunks = (
                [(qs, 512 - qs), (512, 512)] if kt < 4 else [(qs, T - qs)]
            )
            for (q0, w) in chunks:
                sps = ps512.tile([P, 512], F32, tag="ps512")
                nc.tensor.matmul(
                    sps[:, :w],
                    qkT[hb : hb + 64, kj, kt * P : (kt + 1) * P],
                    qkT[hb : hb + 64, qj, q0 : q0 + w],
                    start=True,
                    stop=True,
                )
                nc.scalar.activation(
                    PT[:, kt, q0 : q0 + w],
                    sps[:, :w],
                    mybir.ActivationFunctionType.Exp,
                    scale=0.125,
                )
            nc.vector.tensor_mul(
                PT[:, kt, qs : qs + P], PT[:, kt, qs : qs + P], tri[:]
            )

        # PV + concurrent denominator, then normalize
        for b in range(2):
            yD = ps512.tile([P, 512], F32, tag="ps512")
            den = ps512.tile([P, 512], F32, tag="ps512")
            lo, hi = (0, 4) if b == 0 else (0, 8)
            for kt in range(lo, hi):
                off = max(0, kt * P - b * 512)
                first, last = (kt == lo), (kt == hi - 1)
                rhs = PT[:, kt, b * 512 + off : (b + 1) * 512]
                nc.tensor.matmul(
                    yD[vb : vb + 64, off:512],
                    V[:, kt, h * HS : (h + 1) * HS],
                    rhs,
                    start=first,
                    stop=last,
                    tile_position=(0, vb),
                )
                nc.tensor.matmul(
                    den[db : db + 1, off:512],
                    ones_bf[:, 0:1],
                    rhs,
                    start=first,
                    stop=last,
                    tile_position=(0, db),
                )
            Dr = work.tile([P, 512], F32, tag="Dr")
            nc.vector.reciprocal(Dr[db : db + 1, :], den[db : db + 1, :])
            Drb = work.tile([P, 512], BF16, tag="Drb")
            nc.vector.tensor_copy(Drb[db : db + 1, :], Dr[db : db + 1, :])
            rps = ps512.tile([P, 512], F32, tag="ps512")
            nc.tensor.matmul(
                rps[vb : vb + 64, :],
                ones_bf[db : db + 1, 0:64],
                Drb[db : db + 1, :],
                start=True,
                stop=True,
                tile_position=(db, vb),
            )
            Rh = work.tile([P, 512], F32, tag="Rh")
            nc.scalar.activation(
                Rh[vb : vb + 64, :],
                rps[vb : vb + 64, :],
                mybir.ActivationFunctionType.Copy,
            )
            nc.vector.tensor_mul(
                yT[hb : hb + 64, h // 2, b * 512 : (b + 1) * 512],
                yD[vb : vb + 64, :],
                Rh[vb : vb + 64, :],
            )

    # ---- phase 3: out = y @ Wp^T + b_p
    for tt in range(TT):
        osb = work.tile([P, C], F32, tag="osb")
        for jb in range(2):
            ps = ps384.tile([P, 384], F32, tag="ps384")
            for ct in range(CT):
                nc.tensor.matmul(
                    ps[:],
                    yT[:, ct, tt * P : (tt + 1) * P],
                    WpT[:, ct, jb * 384 : (jb + 1) * 384],
                    start=(ct == 0),
                    stop=(ct == CT - 1),
                )
            nc.vector.tensor_add(
                osb[:, jb * 384 : (jb + 1) * 384],
                ps[:],
                Bp[:, jb * 384 : (jb + 1) * 384],
            )
        nc.sync.dma_start(
            y_d.rearrange("(tt p) c -> tt p c", p=P)[tt], osb[:]
        )


def build_program(loop=1):
    nc = bacc.Bacc("TRN2", target_bir_lowering=False, debug=False)
    x_d = nc.dram_tensor("x", [T, C], F32, kind="ExternalInput").ap()
    wa_d = nc.dram_tensor("W_attn", [3 * C, C], F32, kind="ExternalInput").ap()
    ba_d = nc.dram_tensor("b_attn", [3 * C], F32, kind="ExternalInput").ap()
    wp_d = nc.dram_tensor("W_proj", [C, C], F32, kind="ExternalInput").ap()
    bp_d = nc.dram_tensor("b_proj", [C], F32, kind="ExternalInput").ap()
    y_d = nc.dram_tensor("y", [T, C], F32, kind="ExternalOutput").ap()

    with tile.TileContext(nc) as tc, ExitStack() as ctx:
        const = ctx.enter_context(tc.tile_pool(name="const", bufs=1))
        persist = ctx.enter_context(tc.tile_pool(name="persist", bufs=1))
        nat = ctx.enter_context(tc.tile_pool(name="nat", bufs=3))
        work = ctx.enter_context(tc.tile_pool(name="work", bufs=2))
        ptp = ctx.enter_context(tc.tile_pool(name="ptp", bufs=2))
        ps512 = ctx.enter_context(tc.tile_pool(name="ps512", bufs=6, space="PSUM"))
        ps384 = ctx.enter_context(tc.tile_pool(name="ps384", bufs=2, space="PSUM"))
        pools = (const, persist, nat, work, ptp, ps512, ps384)

        cst = emit_consts(nc, tc, const, ba_d, bp_d)
        loop_cm = tc.For_i(0, loop, 1) if loop > 1 else contextlib.nullcontext()
        with loop_cm:
            emit_body(nc, tc, pools, cst, x_d, wa_d, wp_d, y_d)

    nc.compile()
    return nc


_CACHED_NC = None


def kernel(x, W_attn, b_attn, W_proj, b_proj):
    from concourse.bass_utils import run_bass_kernel_spmd

    global _CACHED_NC
    if _CACHED_NC is None:
        _CACHED_NC = build_program(loop=1)
    nc = _CACHED_NC

    B = x.shape[0]
    assert B == N_CORES
    in_maps = [
        {
            "x": np.ascontiguousarray(x[b], dtype=np.float32),
            "W_attn": np.asarray(W_attn, dtype=np.float32),
            "b_attn": np.asarray(b_attn, dtype=np.float32),
            "W_proj": np.asarray(W_proj, dtype=np.float32),
            "b_proj": np.asarray(b_proj, dtype=np.float32),
        }
        for b in range(B)
    ]
    res = run_bass_kernel_spmd(nc, in_maps, list(range(N_CORES)))
    return np.stack([res.results[b]["y"] for b in range(B)], axis=0)
